# revision 4
# baseline (speedup 1.0000x reference)
"""Conv2D 3x3 (stride 1, pad 1) Trainium2 Bass kernel.

Problem: x (32, 64, 64, 64) NCHW fp32, weight (128, 64, 3, 3) OIHW, bias (128,).
Output: (32, 128, 64, 64).

Strategy: data-parallel over batch across 8 cores (4 images/core). The host
pre-pads each image channel into a 65-col layout (one shared zero column
between adjacent rows serves as both right-pad of row r and left-pad of row
r+1, plus a top/bottom pad row) and converts x/weights to fp16 (rel-err
~3e-4 vs the 2e-2 budget). The host also pre-builds the +65-shifted copy in
partitions 64-127 so every image loads as ONE 128-partition DMA (two
64-partition DMAs to disjoint halves would serialize on the same SDMA
rings at half bandwidth).

On-chip, partitions 0-63 hold the padded channels and partitions 64-127 the
same data shifted down one padded row, so a single K=128 matmul contracts
the ky=0/ky=1 taps at once. ky=2 taps are K=64 matmuls; output tiles are
processed in pairs so tile A's taps run on PE rows 0-63 concurrently with
tile B's on rows 64-127 (row-group disjoint weight copies), which the PE
executes 2-at-a-time. Net: 2 output tiles per 9 matmul slots.

Front-of-kernel critical path: weights are split into two slot groups so the
first 6 taps' weights (ky0/ky1) arrive in one small DMA; image 0 is split
into 3 column chunks on the scalar HWDGE queue so the first tile-pair can
start ~1.5us after kernel start. A short burst of independent matmuls on
vector-memset scratch (NOT gpsimd - its queue is busy with SWDGE descriptor
generation) fills the initial DMA window so the PE activity monitor's
~3.4us busy window elapses as early as possible (HAM un-throttle
1.2 -> 2.4 GHz).

Bias-add fuses into the PSUM->SBUF eviction, alternating scalar/vector
engines; output stores issue on the otherwise-idle sync HWDGE queue so the
tail (last eviction -> last store receipt) stays short. Outputs store as
fp16 and are widened to fp32 on the host.
"""

import numpy as np

import concourse.bass as bass
import concourse.mybir as mybir
import concourse.tile as tile
from concourse import bacc
from concourse.bass_utils import run_bass_kernel_spmd

N_CORES = 8
NIMG = 4  # images per core
C = 64  # input channels
H = W = 64
O = 128  # output channels
PW = 65  # padded row stride (64 data cols + 1 shared zero col)
PH = 66  # padded rows (top pad + 64 + bottom pad)
IMG = PH * PW  # 4290 padded elements per channel per image
XCOLS = 4296  # IMG + slack: lone-tile ky2 reads reach 4292
# Row-aligned PSUM tiles: 10 groups of 6 output rows + 1 of 4 rows, processed
# as 5 pairs + 1 lone tile. Row alignment lets the eviction compact away the
# 1 garbage column per row so the output staging buffer is contiguous.
TILE_ROWS = [6] * 10 + [4]
NQT = len(TILE_ROWS)  # 11
CH0 = 16 * PW  # 1040: image-0 chunk A covers tile pair (0,1)
CH1 = 40 * PW  # 2600: chunk B covers pairs (2,3),(4,5)
WARM_N = 12  # HAM warmup matmuls (N=256 cold ~215ns each)

F16 = mybir.dt.float16
F32 = mybir.dt.float32

_CACHED_NC = None


def build_nc():
    nc = bacc.Bacc()
    x_in = nc.declare_dram_parameter("xp", [NIMG, 2 * C, XCOLS], F16, isOutput=False)
    w_in = nc.declare_dram_parameter("wcat", [2 * C, 12, O], F16, isOutput=False)
    b_in = nc.declare_dram_parameter("bias", [O, 1], F32, isOutput=False)
    out = nc.declare_dram_parameter("out", [NIMG, O, H, W], F16, isOutput=True)

    with tile.TileContext(nc) as tc:
        with (
            tc.tile_pool(name="const", bufs=1) as const_pool,
            tc.tile_pool(name="xp", bufs=4) as x_pool,
            tc.tile_pool(name="osb", bufs=2) as o_pool,
            tc.tile_pool(name="psum", bufs=8, space="PSUM") as psum_pool,
        ):
            wcat = const_pool.tile([2 * C, 12, O], F16)
            bias_t = const_pool.tile([O, 1], F32)
            # Weight slot groups: 0-5 = ky0/ky1 (needed by the first 6 taps
            # of every tile pair), 6-11 = ky2. Two DMAs so group 1 lands
            # ~0.6us sooner and gates the first matmul less.
            nc.sync.dma_start(wcat[:, 0:6, :], w_in[:, 0:6, :])
            nc.sync.dma_start(wcat[:, 6:12, :], w_in[:, 6:12, :])
            nc.gpsimd.dma_start(bias_t[:, :], b_in[:, :])

            # HAM warmup: the PE clock un-throttles (1.2 -> 2.4 GHz) only
            # after ~3.4us of sustained PE activity. Start that clock during
            # the initial DMA window with independent matmuls on memset
            # scratch. memset on the vector engine - gpsimd is busy
            # generating SWDGE descriptors at kernel start.
            wdum = const_pool.tile([128, 256], F16)
            nc.vector.memset(wdum[:, :], 0.0)
            warm_accs = [
                psum_pool.tile([O, 512], F32, tag="acc", name=f"warm{i}")
                for i in range(2)
            ]
            for i in range(WARM_N):
                nc.tensor.matmul(
                    warm_accs[i % 2][:, 0:256], wdum[:, 0:128], wdum[:, :],
                    start=True, stop=True,
                )

            def tap_lo(acc, qt, q0, ky, kx, start, stop):
                """Tap (ky,kx) for one tile on lower PE rows (base copy).
                Weight slots: ky0 -> kx, ky1 -> 3+kx, ky2 -> 6+kx."""
                slot = (kx, 3 + kx, 6 + kx)[ky]
                off = q0 + PW * ky + kx
                nc.tensor.matmul(
                    acc[:, 0:qt],
                    wcat[0:C, slot, :],
                    xt[0:C, off : off + qt],
                    start=start,
                    stop=stop,
                    skip_group_check=True,
                )

            def tap_hi(acc, qt, q0, ky, kx, start, stop):
                """Tap (ky,kx) on upper PE rows: partitions 64-127 hold the
                +65-shifted copy, so base col q0+65*ky+kx lives at
                q0+65*(ky-1)+kx. Weight slots: ky0 -> 3+kx (upper half
                holds ky0 there), ky1 -> kx, ky2 -> 9+kx."""
                slot = (3 + kx, kx, 9 + kx)[ky]
                off = q0 + PW * (ky - 1) + kx
                nc.tensor.matmul(
                    acc[:, 0:qt],
                    wcat[C : 2 * C, slot, :],
                    xt[C : 2 * C, off : off + qt],
                    start=start,
                    stop=stop,
                    skip_group_check=True,
                )

            def pair1(acc, qt, q0, kx):
                """One ky=0/ky=1 paired tap, K=128 across both halves
                (weight slots 0-2 stack ky0-lower with ky1-upper)."""
                nc.tensor.matmul(
                    acc[:, 0:qt],
                    wcat[:, kx, :],
                    xt[0 : 2 * C, q0 + kx : q0 + kx + qt],
                    start=False,
                    stop=(kx == 2),
                    skip_group_check=True,
                )

            def evict(acc, qt, rows, r0, t):
                """PSUM->SBUF + bias, dropping the garbage col per row."""
                av = acc[:, 0:qt].rearrange("p (r c) -> p r c", c=PW)
                ov = osb[:, r0 * W : (r0 + rows) * W].rearrange(
                    "p (r c) -> p r c", c=W
                )
                if t % 2 == 0:
                    nc.scalar.activation(
                        ov[:, :, :],
                        av[:, :, 0:W],
                        mybir.ActivationFunctionType.Identity,
                        bias=bias_t[:, :],
                    )
                else:
                    nc.vector.tensor_scalar_add(
                        ov[:, :, :], av[:, :, 0:W], bias_t[:, 0:1]
                    )

            # All loads issued up front so no load queues behind eviction
            # work on an engine's FIFO. Single 128-partition DMAs (the host
            # pre-built the shifted copy in partitions 64-127). Queue plan:
            # scalar ring drains image-0's 3 chunks first (ring FIFO), sync
            # ring drains wcat then image 1, gpsimd SWDGE trickles images
            # 2-3 (Q7 descriptor emission self-throttles, so it doesn't
            # flood the SDMA engines at T0).
            xts = []
            for m in range(NIMG):
                xt = x_pool.tile([128, XCOLS], F16)
                xts.append(xt)
                if m == 0:
                    nc.scalar.dma_start(xt[:, 0:CH0], x_in[m, :, 0:CH0])
                    nc.scalar.dma_start(xt[:, CH0:CH1], x_in[m, :, CH0:CH1])
                    nc.scalar.dma_start(xt[:, CH1:XCOLS], x_in[m, :, CH1:XCOLS])
                elif m == 1:
                    nc.sync.dma_start(xt[:, :], x_in[m, :, :])
                else:
                    nc.gpsimd.dma_start(xt[:, :], x_in[m, :, :])

            for m in range(NIMG):
                xt = xts[m]
                osb = o_pool.tile([O, H * W], F16)
                # 4-tile units: tile pairs (even, odd) run their taps
                # 2-at-a-time on disjoint PE row groups; 4-tile grouping
                # halves the per-unit-boundary weight-load stalls. Units:
                # tiles (0-3), (4-7), (8,9,10).
                units = [(0, 1, 2, 3), (4, 5, 6, 7), (8, 9, 10)]
                for unit in units:
                    accs = []
                    for t in unit:
                        accs.append(
                            psum_pool.tile([O, 512], F32, tag="acc", name=f"acc{t}")
                        )
                    # tile pairs: the even tile runs ALL 9 taps on the lower
                    # PE rows while the odd tile runs its 9 on the upper
                    # rows, concurrently into different PSUM banks.
                    for i in range(0, len(unit) - 1, 2):
                        ta, tb = unit[i], unit[i + 1]
                        qta, qtb = TILE_ROWS[ta] * PW, TILE_ROWS[tb] * PW
                        for j in range(9):
                            ky, kx = divmod(j, 3)
                            tap_lo(accs[i], qta, 6 * ta * PW, ky, kx, j == 0, j == 8)
                            tap_hi(accs[i + 1], qtb, 6 * tb * PW, ky, kx, j == 0, j == 8)
                    if len(unit) % 2:
                        # lone tile: ky2 taps serial on lower rows, then
                        # ky0/ky1 as full-K stacked pairs
                        t = unit[-1]
                        qt = TILE_ROWS[t] * PW
                        for kx in range(3):
                            tap_lo(accs[-1], qt, 6 * t * PW, 2, kx, kx == 0, False)
                        for kx in range(3):
                            pair1(accs[-1], qt, 6 * t * PW, kx)
                    for i, t in enumerate(unit):
                        rows = TILE_ROWS[t]
                        evict(accs[i], rows * PW, rows, 6 * t, t)
                        # Stores on the sync HWDGE queue (idle mid-kernel;
                        # keeps the scalar engine free for evictions and the
                        # tail short). Last unit stores per-tile.
                        if t in (3, 7):
                            lo, hi = 6 * unit[0], 6 * t + rows
                            nc.sync.dma_start(
                                out[m, :, lo:hi, :],
                                osb[:, lo * W : hi * W].rearrange(
                                    "p (r c) -> p r c", c=W
                                ),
                            )
                        elif t >= 8:
                            lo, hi = 6 * t, 6 * t + rows
                            nc.sync.dma_start(
                                out[m, :, lo:hi, :],
                                osb[:, lo * W : hi * W].rearrange(
                                    "p (r c) -> p r c", c=W
                                ),
                            )

    nc.compile()
    return nc


def _prep_inputs(x, weight, bias):
    x = np.asarray(x, dtype=np.float32)
    n = x.shape[0]
    z = np.zeros((n, C, PH, PW), dtype=np.float16)
    z[:, :, 1 : 1 + H, 1 : 1 + W] = x  # x(i,j) -> row i+1, col j+1
    flat = z.reshape(n, C, IMG)
    xp = np.zeros((n, 2 * C, XCOLS), dtype=np.float16)
    xp[:, 0:C, :IMG] = flat
    xp[:, C : 2 * C, : IMG - PW] = flat[:, :, PW:]  # +65-shifted copy

    w_t = np.asarray(weight, dtype=np.float32).astype(np.float16)
    w_t = w_t.transpose(1, 2, 3, 0)  # [C, ky, kx, O]
    wcat = np.zeros((2 * C, 12, O), dtype=np.float16)
    wcat[0:C, 0:3, :] = w_t[:, 0, :, :]  # ky=0 lower (+ stacked pairs)
    wcat[C : 2 * C, 0:3, :] = w_t[:, 1, :, :]  # ky=1 upper (+ stacked pairs)
    wcat[0:C, 3:6, :] = w_t[:, 1, :, :]  # ky=1, lower-row tiles
    wcat[C : 2 * C, 3:6, :] = w_t[:, 0, :, :]  # ky=0, upper-row tiles
    wcat[0:C, 6:9, :] = w_t[:, 2, :, :]  # ky=2, lower-row tiles
    wcat[C : 2 * C, 9:12, :] = w_t[:, 2, :, :]  # ky=2, upper-row tiles
    b = np.ascontiguousarray(np.asarray(bias, dtype=np.float32).reshape(O, 1))
    return xp, wcat, b


def _in_maps(x, weight, bias):
    xp, wcat, b = _prep_inputs(x, weight, bias)
    return [
        {"xp": xp[i * NIMG : (i + 1) * NIMG], "wcat": wcat, "bias": b}
        for i in range(N_CORES)
    ]


def kernel(x: np.ndarray, weight: np.ndarray, bias: np.ndarray) -> np.ndarray:
    global _CACHED_NC
    if _CACHED_NC is None:
        _CACHED_NC = build_nc()
    res = run_bass_kernel_spmd(_CACHED_NC, _in_maps(x, weight, bias), list(range(N_CORES)))
    return np.concatenate(
        [r["out"].astype(np.float32) for r in res.results], axis=0
    )


def run_profiled(x, weight, bias, tmpdir=None):
    """Dev helper: run with NTFF tracing, return BassKernelResults."""
    global _CACHED_NC
    if _CACHED_NC is None:
        _CACHED_NC = build_nc()
    return run_bass_kernel_spmd(
        _CACHED_NC, _in_maps(x, weight, bias), list(range(N_CORES)),
        trace=True, tmpdir=tmpdir,
    )
